# revision 18
# baseline (speedup 1.0000x reference)
"""Trainium2 Bass kernel for nn_Attention_70514773066385.

Attention with score-history gating bias:
    Q,K,V = x@Wq, x@Wk, x@Wv  (per head)
    raw   = QK^T/sqrt(D) + sigmoid(alpha)*gamma*score_norm(s_prev)
    y     = softmax(raw + causal_mask) @ V @ Wo
    returns (y, raw)

Sharding over 8 NeuronCores: 4 head-groups (3 heads each, Wq/Wk/Wv
column-sharded, Wo row-sharded) x 2 interleaved query-tile sets
(even/odd 128-row tiles, which balances causal-triangle work and makes
the static program identical on every core).  Each core computes raw
and a partial y for its q rows; the host sums the 4 group partials.

Device layouts (T=2048, HID=768, D=64, 3 heads/core, 8 local q-tiles):
  - x arrives pre-transposed (xt [HID,T]) so projections produce
    Q^T/K^T directly ([d, tokens], contraction dim on partitions).
  - scores S = Q^T.T @ K^T accumulate in PSUM [q=128, k=512] tiles;
    evac fused with the gated-norm add (raw = S + A*s_prev + B, A/B
    per-row affine from bn_stats mean/var).
  - softmax without max-subtraction (raw is bounded): exp on ACT with
    accum_out giving the row-sum; causal mask is data-driven
    (k_rel > q_rel -> -1e4) so one program serves both q-tile sets.
  - w rows are normalized then PE-transposed per 128-chunk for the
    AV matmul (ctx^T accumulates over k chunks); y = ctx @ Wo.
"""

import numpy as np

H, D, HID, T = 12, 64, 768, 2048
EPS = 1e-6
N_CORES = 8
NG = 4          # head groups (cores 2g, 2g+1)
HPG = H // NG   # heads per group = 3
P = 128         # partitions / q-tile rows
NEG = -1.0e4    # causal-mask additive (exp underflows to exactly 0)

_cache: dict = {}


def _build_program(av_bf16=True, n_loop=1, use_f32r=True, pool_off=True):
    import concourse.bass as bass
    import concourse.mybir as mybir
    import concourse.tile as tile
    from concourse import bacc

    f32 = mybir.dt.float32
    av_dt = mybir.dt.bfloat16 if av_bf16 else f32

    CK = HID // P            # 6 contraction chunks for projections
    LQ = T // P // 2         # 8 local q tiles per core
    QW = LQ * P              # 1024 q rows per core
    WC = HPG * D             # 192 weight cols per group
    M0 = P                   # first M split (heads 0,1)
    M1 = WC - P              # 64 (head 2)

    nc = bacc.Bacc("TRN2", target_bir_lowering=False, debug=False,
                   num_devices=N_CORES)

    dram = {}
    def din(name, shape):
        dram[name] = nc.dram_tensor(name, shape, f32, kind="ExternalInput").ap()
        return dram[name]
    def dout(name, shape):
        dram[name] = nc.dram_tensor(name, shape, f32, kind="ExternalOutput").ap()
        return dram[name]

    xt = din("xt", [HID, T])
    xqt = din("xqt", [HID, QW])
    wq = din("wq", [HID, WC])      # pre-scaled by 1/sqrt(D) on host
    wk = din("wk", [HID, WC])
    wv = din("wv", [HID, WC])
    wo = din("wo", [WC, HID])
    sp = din("sp", [HPG, QW, T])
    alph = din("alph", [1, HPG])
    gamm = din("gamm", [1, HPG])
    qrel = din("qrel", [P, 1])     # j*128 + row   (fp32)
    kio = din("kio", [2 * P])      # arange(256)   (fp32)
    raw_o = dout("raw_o", [HPG, QW, T])
    y_o = dout("y_o", [QW, HID])

    import contextlib
    with tile.TileContext(nc) as tc:
        with contextlib.ExitStack() as pctx:
            pools = {
                name: pctx.enter_context(
                    tc.tile_pool(name=name, bufs=bufs, space=space))
                for name, bufs, space in [
                    ("singles", 1, "SBUF"), ("ps512", 2, "PSUM"),
                    ("psT", 2, "PSUM"), ("psctx", 2, "PSUM"),
                    ("sp", 3, "SBUF"), ("raw", 2, "SBUF"),
                    ("norm", 2, "SBUF"), ("wrow", 2, "SBUF"),
                    ("wtmp", 3, "SBUF"), ("stat", 4, "SBUF"),
                    ("ybuf", 2, "SBUF"),
                ]}
            if n_loop > 1:
                ET = mybir.EngineType
                with tc.For_i(0, n_loop, 1,
                              hint_engines=(ET.PE, ET.DVE, ET.Activation,
                                            ET.Pool, ET.SP)):
                    _emit_body(tc, nc, bass, mybir, dram, av_dt, CK, LQ, QW,
                               WC, M0, M1, pools)
            else:
                _emit_body(tc, nc, bass, mybir, dram, av_dt, CK, LQ, QW, WC,
                           M0, M1, pools)

    nc.compile()
    return nc


def _emit_body(tc, nc, bass, mybir, dram, av_dt, CK, LQ, QW, WC, M0, M1,
               pools):
    f32 = mybir.dt.float32
    Alu = mybir.AluOpType
    Act = mybir.ActivationFunctionType
    X = mybir.AxisListType.X

    xt, xqt, wq, wk, wv = (dram["xt"], dram["xqt"], dram["wq"],
                           dram["wk"], dram["wv"])
    wo, sp, alph, gamm = dram["wo"], dram["sp"], dram["alph"], dram["gamm"]
    qrel, kio, raw_o, y_o = (dram["qrel"], dram["kio"], dram["raw_o"],
                             dram["y_o"])

    singles = pools["singles"]
    ps512 = pools["ps512"]
    psT = pools["psT"]
    psctx = pools["psctx"]
    sp_pool = pools["sp"]
    raw_pool = pools["raw"]
    norm_pool = pools["norm"]
    wrow_pool = pools["wrow"]
    wtmp_pool = pools["wtmp"]
    stat_pool = pools["stat"]
    y_pool = pools["ybuf"]

    # ---------------- phase 0: constants ----------------
    xt_sb = singles.tile([P, CK, T], f32, tag="xt")
    for c in range(CK):
        nc.sync.dma_start(out=xt_sb[:, c, :], in_=xt[c * P:(c + 1) * P, :])
    xqt_sb = singles.tile([P, CK, QW], f32, tag="xqt")
    for c in range(CK):
        nc.sync.dma_start(out=xqt_sb[:, c, :], in_=xqt[c * P:(c + 1) * P, :])
    wq_sb = singles.tile([P, CK, WC], f32, tag="wq")
    wk_sb = singles.tile([P, CK, WC], f32, tag="wk")
    wv_sb = singles.tile([P, CK, WC], f32, tag="wv")
    for c in range(CK):
        nc.sync.dma_start(out=wq_sb[:, c, :], in_=wq[c * P:(c + 1) * P, :])
        nc.sync.dma_start(out=wk_sb[:, c, :], in_=wk[c * P:(c + 1) * P, :])
        nc.sync.dma_start(out=wv_sb[:, c, :], in_=wv[c * P:(c + 1) * P, :])
    wo1_sb = singles.tile([P, HID], av_dt, tag="wo1")
    wo2_sb = singles.tile([M1, HID], av_dt, tag="wo2")
    if av_dt != f32:
        wo1_f = singles.tile([P, HID], f32, tag="wo1f")
        wo2_f = singles.tile([M1, HID], f32, tag="wo2f")
        nc.sync.dma_start(out=wo1_f, in_=wo[0:P, :])
        nc.sync.dma_start(out=wo2_f, in_=wo[P:WC, :])
        nc.vector.tensor_copy(wo1_sb, wo1_f)
        nc.vector.tensor_copy(wo2_sb, wo2_f)
    else:
        nc.sync.dma_start(out=wo1_sb, in_=wo[0:P, :])
        nc.sync.dma_start(out=wo2_sb, in_=wo[P:WC, :])

    # causal helpers
    qrel_sb = singles.tile([P, 1], f32, tag="qrel")
    nc.sync.dma_start(out=qrel_sb, in_=qrel)
    kio_sb = singles.tile([P, 2 * P], f32, tag="kio")
    kio_b = bass.AP(tensor=kio.tensor, offset=kio.offset,
                    ap=[[0, P], [1, 2 * P]])
    nc.sync.dma_start(out=kio_sb, in_=kio_b)
    mneg_sb = singles.tile([P, 2 * P], f32, tag="mneg")
    # (kio > qrel) * NEG  -> additive causal mask for the diagonal pair
    nc.vector.tensor_scalar(out=mneg_sb, in0=kio_sb, scalar1=qrel_sb,
                            scalar2=NEG, op0=Alu.is_gt, op1=Alu.mult)

    # gate g_h = sigmoid(alpha_h) * gamma_h, broadcast to [P,1] per head
    ga_sb = singles.tile([1, HPG], f32, tag="ga")
    gm_sb = singles.tile([1, HPG], f32, tag="gm")
    nc.sync.dma_start(out=ga_sb, in_=alph)
    nc.sync.dma_start(out=gm_sb, in_=gamm)
    gv_sb = singles.tile([1, HPG], f32, tag="gv")
    nc.scalar.activation(out=gv_sb, in_=ga_sb, func=Act.Sigmoid)
    nc.vector.tensor_mul(gv_sb, gv_sb, gm_sb)
    gb_sb = singles.tile([P, HPG], f32, tag="gb")
    for h in range(HPG):
        nc.gpsimd.partition_broadcast(gb_sb[:, h:h + 1], gv_sb[0:1, h:h + 1])

    eps_sb = singles.tile([P, 1], f32, tag="eps")
    nc.vector.memset(eps_sb, EPS)
    ident = singles.tile([P, P], av_dt, tag="ident")
    from concourse.masks import make_identity
    make_identity(nc, ident)

    # ---------------- phase 1: projections ----------------
    # Q^T/K^T: out[d_cols, tokens] = W.T @ x^T ; M splits [0:128],[128:192]
    qt01 = singles.tile([P, QW], f32, tag="qt01")
    qt2 = singles.tile([M1, QW], f32, tag="qt2")
    kt01 = singles.tile([P, T], f32, tag="kt01")
    kt2 = singles.tile([M1, T], f32, tag="kt2")

    def project_T(w_sb, x_sb, width, out01, out2):
        for n0 in range(0, width, 512):
            nw = min(512, width - n0)
            for mi, (mof, msz, outt) in enumerate(
                    [(0, M0, out01), (M0, M1, out2)]):
                ps = ps512.tile([P, 512], f32, tag="ps512")
                for c in range(CK):
                    nc.tensor.matmul(
                        ps[0:msz, 0:nw], w_sb[:, c, mof:mof + msz],
                        x_sb[:, c, n0:n0 + nw],
                        start=(c == 0), stop=(c == CK - 1))
                nc.scalar.copy(outt[:, n0:n0 + nw], ps[0:msz, 0:nw])

    project_T(wq_sb, xqt_sb, QW, qt01, qt2)
    project_T(wk_sb, xt_sb, T, kt01, kt2)

    # V: natural layout [tokens, d] ; lhsT = xt chunk (stationary)
    v_sb = singles.tile([P, T // P, WC], av_dt, tag="v")
    for tk in range(T // P):
        ps = ps512.tile([P, 512], f32, tag="ps512")
        for c in range(CK):
            nc.tensor.matmul(ps[:, 0:WC], xt_sb[:, c, tk * P:(tk + 1) * P],
                             wv_sb[:, c, :],
                             start=(c == 0), stop=(c == CK - 1))
        nc.scalar.copy(v_sb[:, tk, :], ps[:, 0:WC])

    # ---------------- phase 2: attention ----------------
    ctx01 = singles.tile([P, QW], av_dt, tag="ctx01")
    ctx2 = singles.tile([M1, QW], av_dt, tag="ctx2")

    for t in range(LQ):
        nkc = 2 * t + 2            # k chunks up to & incl. diagonal pair
        dg0 = 2 * t * P            # diagonal pair column offset
        for h in range(HPG):
            sp_t = sp_pool.tile([P, T], f32, tag="sp")
            nc.sync.dma_start(out=sp_t, in_=sp[h, t * P:(t + 1) * P, :])

            # row stats -> A = g*rstd, B = -mean*A
            nsg = max(1, T // 512)
            stats = stat_pool.tile([P, nsg, 6], f32, tag="stats")
            for s4 in range(nsg):
                nc.vector.bn_stats(out=stats[:, s4, :],
                                   in_=sp_t[:, s4 * 512:(s4 + 1) * 512])
            mv = stat_pool.tile([P, 2], f32, tag="mv")
            nc.vector.bn_aggr(out=mv, in_=stats)
            sd = stat_pool.tile([P, 1], f32, tag="sd")
            nc.scalar.activation(out=sd, in_=mv[:, 1:2], func=Act.Sqrt,
                                 bias=eps_sb, scale=1.0)
            rec_sd = stat_pool.tile([P, 1], f32, tag="rsd")
            nc.vector.reciprocal(rec_sd, sd)
            A = stat_pool.tile([P, 1], f32, tag="A")
            nc.vector.tensor_mul(A, rec_sd, gb_sb[:, h:h + 1])
            B = stat_pool.tile([P, 1], f32, tag="B")
            nc.vector.tensor_mul(B, mv[:, 0:1], A)
            nc.vector.tensor_scalar_mul(B, B, -1.0)

            # scores + gated norm -> raw
            if h < 2:
                lhsT = qt01[h * D:(h + 1) * D, t * P:(t + 1) * P]
                kth = kt01
                kof = h * D
            else:
                lhsT = qt2[0:D, t * P:(t + 1) * P]
                kth = kt2
                kof = 0
            raw_t = raw_pool.tile([P, T], f32, tag="raw")
            for ks in range(max(1, T // 512)):
                pss = ps512.tile([P, 512], f32, tag="ps512")
                nc.tensor.matmul(pss, lhsT,
                                 kth[kof:kof + D, ks * 512:(ks + 1) * 512],
                                 start=True, stop=True)
                nt = norm_pool.tile([P, 512], f32, tag="norm")
                nc.vector.tensor_scalar(out=nt, in0=sp_t[:, ks * 512:(ks + 1) * 512],
                                        scalar1=A, scalar2=B,
                                        op0=Alu.mult, op1=Alu.add)
                nc.vector.tensor_add(raw_t[:, ks * 512:(ks + 1) * 512], pss, nt)
            nc.sync.dma_start(out=raw_o[h, t * P:(t + 1) * P, :], in_=raw_t)

            # exp (+ causal mask on the diagonal pair), denominator
            wrow = wrow_pool.tile([P, T], av_dt, tag="wrow")
            den_p = stat_pool.tile([P, 6], f32, tag="denp")
            nsp = 0
            for c0 in range(0, dg0, 512):
                w = min(512, dg0 - c0)
                nc.scalar.activation(out=wrow[:, c0:c0 + w],
                                     in_=raw_t[:, c0:c0 + w], func=Act.Exp,
                                     accum_out=den_p[:, nsp:nsp + 1])
                nsp += 1
            dgm = raw_pool.tile([P, 2 * P], f32, tag="dgm")
            nc.vector.tensor_add(dgm, raw_t[:, dg0:dg0 + 2 * P], mneg_sb)
            nc.scalar.activation(out=wrow[:, dg0:dg0 + 2 * P], in_=dgm,
                                 func=Act.Exp, accum_out=den_p[:, nsp:nsp + 1])
            nsp += 1
            den = stat_pool.tile([P, 1], f32, tag="den")
            nc.vector.tensor_reduce(out=den, in_=den_p[:, 0:nsp], axis=X,
                                    op=Alu.add)
            rec = stat_pool.tile([P, 1], f32, tag="rec")
            nc.vector.reciprocal(rec, den)
            nc.vector.tensor_scalar_mul(wrow[:, 0:nkc * P], wrow[:, 0:nkc * P],
                                        rec)

            # transpose w chunks; AV accumulate ctx^T[d, qtile]
            cps = psctx.tile([D, P], f32, tag="psctx")
            for kc in range(nkc):
                pt = psT.tile([P, P], av_dt, tag="psT")
                nc.tensor.transpose(pt, wrow[:, kc * P:(kc + 1) * P], ident)
                wt = wtmp_pool.tile([P, P], av_dt, tag="wtmp")
                nc.vector.tensor_copy(wt, pt)
                nc.tensor.matmul(cps, v_sb[:, kc, h * D:(h + 1) * D], wt,
                                 start=(kc == 0), stop=(kc == nkc - 1))
            dst = ctx01[h * D:(h + 1) * D, t * P:(t + 1) * P] if h < 2 \
                else ctx2[0:D, t * P:(t + 1) * P]
            nc.scalar.copy(dst, cps)

    # ---------------- phase 3: output projection ----------------
    for t in range(LQ):
        psy = ps512.tile([P, HID], f32, tag="ps512")
        for od in range(0, HID, 512):
            w = min(512, HID - od)
            nc.tensor.matmul(psy[:, od:od + w],
                             ctx01[:, t * P:(t + 1) * P],
                             wo1_sb[:, od:od + w], start=True, stop=False)
            nc.tensor.matmul(psy[:, od:od + w],
                             ctx2[:, t * P:(t + 1) * P],
                             wo2_sb[:, od:od + w], start=False, stop=True)
        yb = y_pool.tile([P, HID], f32, tag="ybuf")
        nc.vector.tensor_copy(yb, psy)
        nc.sync.dma_start(out=y_o[t * P:(t + 1) * P, :], in_=yb)


# ======================= host side =======================

def _sigmoid(x):
    return 1.0 / (1.0 + np.exp(-x))


def _shard_inputs(x, s_prev, Wq, Wk, Wv, Wo, alpha, gamma):
    """Build the 8 per-core input dicts."""
    x2 = np.ascontiguousarray(x.reshape(T, HID))
    xt_full = np.ascontiguousarray(x2.T)                    # [HID, T]
    sqd = np.float32(1.0 / np.sqrt(D))
    in_maps = []
    LQ = T // P // 2
    for c in range(N_CORES):
        g, j = c // 2, c % 2
        hs = slice(g * HPG, (g + 1) * HPG)
        cols = slice(g * HPG * D, (g + 1) * HPG * D)
        # local q rows: global tiles 2t+j
        spg = s_prev[0, hs].reshape(HPG, LQ, 2, P, T)[:, :, j]   # [3, 8, 128, T]
        sp_c = np.ascontiguousarray(spg).reshape(HPG, LQ * P, T)
        xq = x2.reshape(LQ, 2, P, HID)[:, j].reshape(LQ * P, HID)
        xqt = np.ascontiguousarray(xq.T)
        qrel_v = (j * P + np.arange(P, dtype=np.float32)).reshape(P, 1)
        in_maps.append({
            "xt": xt_full,
            "xqt": xqt,
            "wq": np.ascontiguousarray(Wq[:, cols]) * sqd,
            "wk": np.ascontiguousarray(Wk[:, cols]),
            "wv": np.ascontiguousarray(Wv[:, cols]),
            "wo": np.ascontiguousarray(Wo[cols, :]),
            "sp": sp_c,
            "alph": alpha[hs].reshape(1, HPG).astype(np.float32),
            "gamm": gamma[hs].reshape(1, HPG).astype(np.float32),
            "qrel": qrel_v,
            "kio": np.arange(2 * P, dtype=np.float32),
        })
    return in_maps


def _unshard(results):
    LQ = T // P // 2
    raw = np.empty((1, H, T, T), np.float32)
    y = np.zeros((1, T, HID), np.float32)
    for c in range(N_CORES):
        g, j = c // 2, c % 2
        r = results[c]["raw_o"].reshape(HPG, LQ, P, T)
        raw[0, g * HPG:(g + 1) * HPG].reshape(HPG, LQ, 2, P, T)[:, :, j] = r
        yv = y[0].reshape(LQ, 2, P, HID)[:, j]
        yv += results[c]["y_o"].reshape(LQ, P, HID)
    return y, raw


class _Runner:
    """Persistent-jit shard_map executor (axon/PJRT path)."""

    def __init__(self, nc):
        import jax
        from jax.sharding import Mesh, PartitionSpec
        from jax.experimental.shard_map import shard_map
        from concourse import bass2jax
        import concourse.mybir as mybir

        bass2jax.install_neuronx_cc_hook()
        self.jax = jax
        pname = nc.partition_id_tensor.name if nc.partition_id_tensor else None
        in_names, out_names, out_avals = [], [], []
        for alloc in nc.m.functions[0].allocations:
            if not isinstance(alloc, mybir.MemoryLocationSet):
                continue
            name = alloc.memorylocations[0].name
            if alloc.kind == "ExternalInput":
                if name != pname:
                    in_names.append(name)
            elif alloc.kind == "ExternalOutput":
                out_avals.append(jax.core.ShapedArray(
                    tuple(alloc.tensor_shape), mybir.dt.np(alloc.dtype)))
                out_names.append(name)
        self.in_names, self.out_names, self.out_avals = \
            in_names, out_names, out_avals
        n_params, n_outs = len(in_names), len(out_names)
        all_in = list(in_names) + list(out_names)
        if pname is not None:
            all_in.append(pname)

        def _body(*args):
            operands = list(args)
            if pname is not None:
                operands.append(bass2jax.partition_id_tensor())
            return tuple(bass2jax._bass_exec_p.bind(
                *operands, out_avals=tuple(out_avals),
                in_names=tuple(all_in), out_names=tuple(out_names),
                lowering_input_output_aliases=(),
                sim_require_finite=True, sim_require_nnan=True, nc=nc))

        devices = jax.devices()[:N_CORES]
        mesh = Mesh(np.asarray(devices), ("core",))
        specs = (PartitionSpec("core"),)
        self.fn = jax.jit(
            shard_map(_body, mesh=mesh,
                      in_specs=specs * (n_params + n_outs),
                      out_specs=specs * n_outs, check_rep=False),
            keep_unused=True)
        self.n_params = n_params
        self.zero_shapes = [tuple(a.shape) for a in out_avals]
        self.zero_dtypes = [a.dtype for a in out_avals]

    def prepare(self, in_maps):
        n = N_CORES
        concat_in = [
            np.concatenate([np.asarray(m[nm]) for m in in_maps], axis=0)
            for nm in self.in_names]
        concat_zero = [np.zeros((n * s[0], *s[1:]), d)
                       for s, d in zip(self.zero_shapes, self.zero_dtypes)]
        return [self.jax.device_put(a) for a in concat_in + concat_zero]

    def run(self, args):
        outs = self.fn(*args)
        self.jax.block_until_ready(outs)
        return outs

    def __call__(self, in_maps):
        n = N_CORES
        outs = self.run(self.prepare(in_maps))
        return [
            {nm: np.asarray(outs[i]).reshape(n, *self.out_avals[i].shape)[c]
             for i, nm in enumerate(self.out_names)}
            for c in range(n)]


def _get_runner():
    if "runner" not in _cache:
        nc = _build_program()
        _cache["runner"] = _Runner(nc)
    return _cache["runner"]


def measure_hw_ns(np_inputs, n_lo=17, n_hi=257, calls=8):
    """Device-side per-pass time from the slope between two in-NEFF
    For_i loop variants: (wall(n_hi) - wall(n_lo)) / (n_hi - n_lo)."""
    import time
    in_maps = _shard_inputs(
        np.asarray(np_inputs["x"], np.float32),
        np.asarray(np_inputs["s_prev"], np.float32),
        np.asarray(np_inputs["Wq"], np.float32),
        np.asarray(np_inputs["Wk"], np.float32),
        np.asarray(np_inputs["Wv"], np.float32),
        np.asarray(np_inputs["Wo"], np.float32),
        np.asarray(np_inputs["alpha"], np.float32),
        np.asarray(np_inputs["gamma"], np.float32))

    def _timed(runner):
        args = runner.prepare(in_maps)
        runner.run(args)  # warm
        best = float("inf")
        for _ in range(calls):
            t0 = time.perf_counter()
            runner.run(args)
            best = min(best, time.perf_counter() - t0)
        return best

    walls = {}
    for n in (n_lo, n_hi):
        key = f"runner_loop{n}"
        if key not in _cache:
            _cache[key] = _Runner(_build_program(n_loop=n))
        walls[n] = _timed(_cache[key])
        print(f"  wall {n}x: {walls[n]*1e3:.2f} ms")
    return (walls[n_hi] - walls[n_lo]) / (n_hi - n_lo) * 1e9


def kernel(x, s_prev, Wq, Wk, Wv, Wo, alpha, gamma):
    x = np.asarray(x, np.float32)
    s_prev = np.asarray(s_prev, np.float32)
    in_maps = _shard_inputs(x, s_prev,
                            np.asarray(Wq, np.float32),
                            np.asarray(Wk, np.float32),
                            np.asarray(Wv, np.float32),
                            np.asarray(Wo, np.float32),
                            np.asarray(alpha, np.float32),
                            np.asarray(gamma, np.float32))
    results = _get_runner()(in_maps)
    return _unshard(results)


# revision 30
# speedup vs baseline: 1.4439x; 1.4439x over previous
"""Trainium2 Bass kernel for nn_Attention_70514773066385.

Attention with score-history gating bias:
    Q,K,V = x@Wq, x@Wk, x@Wv  (per head)
    raw   = QK^T/sqrt(D) + sigmoid(alpha)*gamma*score_norm(s_prev)
    y     = softmax(raw + causal_mask) @ V @ Wo
    returns (y, raw)

Sharding over 8 NeuronCores: 4 head-groups (3 heads each, Wq/Wk/Wv
column-sharded, Wo row-sharded) x 2 interleaved query-tile sets
(even/odd 128-row tiles, which balances causal-triangle work and makes
the static program identical on every core).  Each core computes raw
and a partial y for its q rows; the host sums the 4 group partials.

Device layouts (T=2048, HID=768, D=64, 3 heads/core, 8 local q-tiles):
  - x arrives pre-transposed (xt [HID,T]) so projections produce
    Q^T/K^T directly ([d, tokens], contraction dim on partitions).
  - scores S = Q^T.T @ K^T accumulate in PSUM [q=128, k=512] tiles;
    evac fused with the gated-norm add (raw = S + A*s_prev + B, A/B
    per-row affine from bn_stats mean/var).
  - softmax without max-subtraction (raw is bounded): exp on ACT with
    accum_out giving the row-sum; causal mask is data-driven
    (k_rel > q_rel -> -1e4) so one program serves both q-tile sets.
  - w rows are normalized then PE-transposed per 128-chunk for the
    AV matmul (ctx^T accumulates over k chunks); y = ctx @ Wo.
"""

import numpy as np

H, D, HID, T = 12, 64, 768, 2048
EPS = 1e-6
N_CORES = 8
NG = 4          # head groups (cores 2g, 2g+1)
HPG = H // NG   # heads per group = 3
P = 128         # partitions / q-tile rows
NEG = -1.0e4    # causal-mask additive (exp underflows to exactly 0)

_cache: dict = {}


def _build_program(av_bf16=True, n_loop=1, use_f32r=True, pool_off=True, defer_av=False):
    import concourse.bass as bass
    import concourse.mybir as mybir
    import concourse.tile as tile
    from concourse import bacc

    f32 = mybir.dt.float32
    av_dt = mybir.dt.bfloat16 if av_bf16 else f32

    CK = HID // P            # 6 contraction chunks for projections
    LQ = T // P // 2         # 8 local q tiles per core
    QW = LQ * P              # 1024 q rows per core
    WC = HPG * D             # 192 weight cols per group
    M0 = P                   # first M split (heads 0,1)
    M1 = WC - P              # 64 (head 2)

    nc = bacc.Bacc("TRN2", target_bir_lowering=False, debug=False,
                   num_devices=N_CORES)

    dram = {}
    def din(name, shape):
        dram[name] = nc.dram_tensor(name, shape, f32, kind="ExternalInput").ap()
        return dram[name]
    def dout(name, shape):
        dram[name] = nc.dram_tensor(name, shape, f32, kind="ExternalOutput").ap()
        return dram[name]

    xt = din("xt", [HID, T])
    xqt = din("xqt", [HID, QW])
    wq = din("wq", [HID, WC])      # pre-scaled by 1/sqrt(D) on host
    wk = din("wk", [HID, WC])
    wv = din("wv", [HID, WC])
    wo = din("wo", [WC, HID])
    sp = din("sp", [HPG, QW, T])
    alph = din("alph", [1, HPG])
    gamm = din("gamm", [1, HPG])
    qrel = din("qrel", [P, 1])     # j*128 + row   (fp32)
    kio = din("kio", [2 * P])      # arange(256)   (fp32)
    raw_o = dout("raw_o", [HPG, QW, T])
    y_o = dout("y_o", [QW, HID])

    import contextlib
    with tile.TileContext(nc) as tc:
        with contextlib.ExitStack() as pctx:
            pools = {
                name: pctx.enter_context(
                    tc.tile_pool(name=name, bufs=bufs, space=space))
                for name, bufs, space in [
                    ("singles", 1, "SBUF"), ("share", 3, "SBUF"),
                    ("ps512", 2, "PSUM"),
                    ("psT", 2, "PSUM"), ("psctx", 2, "PSUM"),
                    ("sp", 3, "SBUF"), ("raw", 2, "SBUF"),
                    ("norm", 2, "SBUF"), ("wrow", 2, "SBUF"),
                    ("wtmp", 3, "SBUF"), ("stat", 4, "SBUF"),
                    ("ybuf", 2, "SBUF"),
                ]}
            if n_loop > 1:
                ET = mybir.EngineType
                with tc.For_i(0, n_loop, 1,
                              hint_engines=(ET.PE, ET.DVE, ET.Activation,
                                            ET.Pool, ET.SP)):
                    _emit_body(tc, nc, bass, mybir, dram, av_dt, CK, LQ, QW,
                               WC, M0, M1, pools, use_f32r, pool_off,
                               defer_av)
            else:
                _emit_body(tc, nc, bass, mybir, dram, av_dt, CK, LQ, QW, WC,
                           M0, M1, pools, use_f32r, pool_off, defer_av)

    nc.compile()
    return nc


def _emit_body(tc, nc, bass, mybir, dram, av_dt, CK, LQ, QW, WC, M0, M1,
               pools, use_f32r=False, pool_off=False, defer_av=False):
    f32 = mybir.dt.float32
    mm_dt = f32
    mmdma = nc.sync
    R = (lambda ap: ap.bitcast(mybir.dt.float32r)) if use_f32r \
        else (lambda ap: ap)
    aff_eng = nc.gpsimd if pool_off else nc.vector
    Alu = mybir.AluOpType
    Act = mybir.ActivationFunctionType
    X = mybir.AxisListType.X

    xt, xqt, wq, wk, wv = (dram["xt"], dram["xqt"], dram["wq"],
                           dram["wk"], dram["wv"])
    wo, sp, alph, gamm = dram["wo"], dram["sp"], dram["alph"], dram["gamm"]
    qrel, kio, raw_o, y_o = (dram["qrel"], dram["kio"], dram["raw_o"],
                             dram["y_o"])

    singles = pools["singles"]
    share_pool = pools["share"]
    ps512 = pools["ps512"]
    psT = pools["psT"]
    psctx = pools["psctx"]
    sp_pool = pools["sp"]
    raw_pool = pools["raw"]
    norm_pool = pools["norm"]
    wrow_pool = pools["wrow"]
    wtmp_pool = pools["wtmp"]
    stat_pool = pools["stat"]
    y_pool = pools["ybuf"]

    # ---------------- phase 0: constants ----------------
    xt_sb = share_pool.tile([P, CK, T], mm_dt, tag="share") if defer_av \
        else singles.tile([P, CK, T], mm_dt, tag="xt")
    for c in range(CK):
        mmdma.dma_start(out=xt_sb[:, c, :], in_=xt[c * P:(c + 1) * P, :])
    xqt_sb = share_pool.tile([P, CK, QW], mm_dt, tag="share") if defer_av \
        else singles.tile([P, CK, QW], mm_dt, tag="xqt")
    for c in range(CK):
        mmdma.dma_start(out=xqt_sb[:, c, :], in_=xqt[c * P:(c + 1) * P, :])
    wqkv_sb = share_pool.tile([P, CK, 3 * WC], mm_dt, tag="share") if defer_av \
        else singles.tile([P, CK, 3 * WC], mm_dt, tag="wqkv")
    wq_sb = wqkv_sb[:, :, 0:WC]
    wk_sb = wqkv_sb[:, :, WC:2 * WC]
    wv_sb = wqkv_sb[:, :, 2 * WC:3 * WC]
    for c in range(CK):
        mmdma.dma_start(out=wq_sb[:, c, :], in_=wq[c * P:(c + 1) * P, :])
        mmdma.dma_start(out=wk_sb[:, c, :], in_=wk[c * P:(c + 1) * P, :])
        mmdma.dma_start(out=wv_sb[:, c, :], in_=wv[c * P:(c + 1) * P, :])
    wo1_sb = singles.tile([P, HID], av_dt, tag="wo1")
    wo2_sb = singles.tile([M1, HID], av_dt, tag="wo2")
    wo_dma = nc.gpsimd if av_dt != f32 else nc.sync
    wo_dma.dma_start(out=wo1_sb, in_=wo[0:P, :])
    wo_dma.dma_start(out=wo2_sb, in_=wo[P:WC, :])

    # causal helpers
    qrel_sb = singles.tile([P, 1], f32, tag="qrel")
    nc.sync.dma_start(out=qrel_sb, in_=qrel)
    kio_sb = norm_pool.tile([P, 2 * P], f32, tag="norm")
    kio_b = bass.AP(tensor=kio.tensor, offset=kio.offset,
                    ap=[[0, P], [1, 2 * P]])
    nc.sync.dma_start(out=kio_sb, in_=kio_b)
    mneg_sb = singles.tile([P, 2 * P], f32, tag="mneg")
    # (kio > qrel) * NEG  -> additive causal mask for the diagonal pair
    nc.vector.tensor_scalar(out=mneg_sb, in0=kio_sb, scalar1=qrel_sb,
                            scalar2=NEG, op0=Alu.is_gt, op1=Alu.mult)

    # gate g_h = sigmoid(alpha_h) * gamma_h, broadcast to [P,1] per head
    ga_sb = singles.tile([1, HPG], f32, tag="ga")
    gm_sb = singles.tile([1, HPG], f32, tag="gm")
    nc.sync.dma_start(out=ga_sb, in_=alph)
    nc.sync.dma_start(out=gm_sb, in_=gamm)
    gv_sb = singles.tile([1, HPG], f32, tag="gv")
    nc.scalar.activation(out=gv_sb, in_=ga_sb, func=Act.Sigmoid)
    nc.vector.tensor_mul(gv_sb, gv_sb, gm_sb)
    gb_sb = singles.tile([P, HPG], f32, tag="gb")
    for h in range(HPG):
        nc.gpsimd.partition_broadcast(gb_sb[:, h:h + 1], gv_sb[0:1, h:h + 1])

    eps_sb = singles.tile([P, 1], f32, tag="eps")
    nc.vector.memset(eps_sb, EPS)
    ident = singles.tile([P, P], av_dt, tag="ident")
    from concourse.masks import make_identity
    make_identity(nc, ident)

    # ---------------- phase 1: projections ----------------
    # Q^T/K^T: out[d_cols, tokens] = W.T @ x^T ; M splits [0:128],[128:192]
    qt01 = singles.tile([P, QW], mm_dt, tag="qt01")
    qt2 = singles.tile([M1, QW], mm_dt, tag="qt2")
    kt01 = singles.tile([P, T], mm_dt, tag="kt01")
    kt2 = singles.tile([M1, T], mm_dt, tag="kt2")

    def project_T(w_sb, x_sb, width, out01, out2):
        for n0 in range(0, width, 512):
            nw = min(512, width - n0)
            for mi, (mof, msz, outt) in enumerate(
                    [(0, M0, out01), (M0, M1, out2)]):
                ps = ps512.tile([P, 512], f32, tag="ps512")
                for c in range(CK):
                    nc.tensor.matmul(
                        ps[0:msz, 0:nw], R(w_sb[:, c, mof:mof + msz]),
                        R(x_sb[:, c, n0:n0 + nw]),
                        start=(c == 0), stop=(c == CK - 1))
                nc.scalar.copy(outt[:, n0:n0 + nw], ps[0:msz, 0:nw])

    project_T(wq_sb, xqt_sb, QW, qt01, qt2)
    project_T(wk_sb, xt_sb, T, kt01, kt2)

    # V: natural layout [tokens, d] ; lhsT = xt chunk (stationary).
    # Emission is spread into the t loop (two token tiles per iteration)
    # so phase-1 PE work does not cork the s_prev/score pipeline.
    v_sb = singles.tile([P, T // P, WC], av_dt, tag="v")

    def emit_v(tk):
        ps = ps512.tile([P, 1024], f32, tag="ps512", name="psv")
        for c in range(CK):
            nc.tensor.matmul(ps[:, 0:WC],
                             R(xt_sb[:, c, tk * P:(tk + 1) * P]),
                             R(wv_sb[:, c, :]),
                             start=(c == 0), stop=(c == CK - 1))
        nc.scalar.copy(v_sb[:, tk, :], ps[:, 0:WC])

    # ---------------- phase 2: attention ----------------
    ctx01 = singles.tile([P, QW], av_dt, tag="ctx01")
    ctx2 = singles.tile([M1, QW], av_dt, tag="ctx2")

    # ragged store for transposed-normalized w chunks: for k-chunk kc the
    # q tiles t >= tmin(kc) participate (tile t covers chunks <= 2t+1)
    NKC = T // P
    wt_tmin = [kc // 2 for kc in range(NKC)]
    wt_off = []
    _o = 0
    for kc in range(NKC):
        wt_off.append(_o)
        _o += (LQ - wt_tmin[kc]) * P
    wt_sb = [share_pool.tile([P, _o], av_dt, tag="share", name=f"wt{h}")
             for h in range(HPG)] if defer_av else None

    for t in range(LQ):
        nkc = 2 * t + 2            # k chunks up to & incl. diagonal pair
        dg0 = 2 * t * P            # diagonal pair column offset
        emit_v(2 * t)
        emit_v(2 * t + 1)

        # stats for all heads of this tile first (one Sqrt -> one LUT swap)
        nsg = max(1, T // 512)
        sp_ts = []
        mv3 = stat_pool.tile([P, HPG, 2], f32, tag="mv3")
        for h in range(HPG):
            sp_t = sp_pool.tile([P, T], f32, tag="sp")
            nc.sync.dma_start(out=sp_t, in_=sp[h, t * P:(t + 1) * P, :])
            sp_ts.append(sp_t)
            stats = stat_pool.tile([P, nsg, 6], f32, tag="stats")
            for s4 in range(nsg):
                nc.vector.bn_stats(out=stats[:, s4, :],
                                   in_=sp_t[:, s4 * 512:(s4 + 1) * 512])
            nc.vector.bn_aggr(out=mv3[:, h, :], in_=stats)
        sd3 = stat_pool.tile([P, HPG], f32, tag="sd3")
        nc.scalar.activation(out=sd3, in_=mv3[:, :, 1], func=Act.Sqrt,
                             bias=eps_sb, scale=1.0)
        A3 = stat_pool.tile([P, HPG], f32, tag="A3")
        nc.vector.reciprocal(A3, sd3)
        nc.vector.tensor_mul(A3, A3, gb_sb)
        B3 = stat_pool.tile([P, HPG], f32, tag="B3")
        nc.vector.tensor_mul(B3, mv3[:, :, 0], A3)
        nc.vector.tensor_scalar_mul(B3, B3, -1.0)

        for h in range(HPG):
            sp_t = sp_ts[h]
            A = A3[:, h:h + 1]
            B = B3[:, h:h + 1]
            # scores + gated norm -> raw
            if h < 2:
                lhsT = qt01[h * D:(h + 1) * D, t * P:(t + 1) * P]
                kth = kt01
                kof = h * D
            else:
                lhsT = qt2[0:D, t * P:(t + 1) * P]
                kth = kt2
                kof = 0
            raw_t = raw_pool.tile([P, T], f32, tag="raw")
            for kb in range(0, T, 1024):
                kw = min(1024, T - kb)
                pss = ps512.tile([P, 1024], f32, tag="ps512")
                for ks in range(kb, kb + kw, 512):
                    nc.tensor.matmul(pss[:, ks - kb:ks - kb + 512], R(lhsT),
                                     R(kth[kof:kof + D, ks:ks + 512]),
                                     start=True, stop=True)
                nt = norm_pool.tile([P, 1024], f32, tag="norm")
                aff_eng.tensor_scalar(out=nt[:, 0:kw],
                                      in0=sp_t[:, kb:kb + kw],
                                      scalar1=A, scalar2=B,
                                      op0=Alu.mult, op1=Alu.add)
                nc.vector.tensor_add(raw_t[:, kb:kb + kw], pss[:, 0:kw],
                                     nt[:, 0:kw])
            nc.sync.dma_start(out=raw_o[h, t * P:(t + 1) * P, :], in_=raw_t)

            # exp (+ causal mask on the diagonal pair), denominator
            wrow = wrow_pool.tile([P, T], av_dt, tag="wrow")
            den_p = stat_pool.tile([P, 6], f32, tag="denp")
            nsp = 0
            for c0 in range(0, dg0, 512):
                w = min(512, dg0 - c0)
                nc.scalar.activation(out=wrow[:, c0:c0 + w],
                                     in_=raw_t[:, c0:c0 + w], func=Act.Exp,
                                     accum_out=den_p[:, nsp:nsp + 1])
                nsp += 1
            dgm = raw_pool.tile([P, 2 * P], f32, tag="dgm")
            nc.vector.tensor_add(dgm, raw_t[:, dg0:dg0 + 2 * P], mneg_sb)
            nc.scalar.activation(out=wrow[:, dg0:dg0 + 2 * P], in_=dgm,
                                 func=Act.Exp, accum_out=den_p[:, nsp:nsp + 1])
            nsp += 1
            den = stat_pool.tile([P, 1], f32, tag="den")
            nc.vector.tensor_reduce(out=den, in_=den_p[:, 0:nsp], axis=X,
                                    op=Alu.add)
            rec = stat_pool.tile([P, 1], f32, tag="rec")
            nc.vector.reciprocal(rec, den)
            nc.vector.tensor_scalar_mul(wrow[:, 0:nkc * P],
                                        wrow[:, 0:nkc * P], rec)

            if defer_av:
                # transpose w chunks into the per-head ragged store
                for kc in range(nkc):
                    pt = psT.tile([P, P], av_dt, tag="psT")
                    nc.tensor.transpose(pt, wrow[:, kc * P:(kc + 1) * P],
                                        ident)
                    col = wt_off[kc] + (t - wt_tmin[kc]) * P
                    nc.vector.tensor_copy(wt_sb[h][:, col:col + P], pt)
            else:
                # transpose in groups of <=4 chunks into one PSUM tile,
                # evacuate with a single wide DVE copy, then AV
                cps = psctx.tile([D, P], f32, tag="psctx")
                kc = 0
                for g0 in range(0, nkc, 4):
                    gn = min(4, nkc - g0)
                    pt = psT.tile([P, 4 * P], av_dt, tag="psT")
                    for gi in range(gn):
                        nc.tensor.transpose(
                            pt[:, gi * P:(gi + 1) * P],
                            wrow[:, (g0 + gi) * P:(g0 + gi + 1) * P], ident)
                    wt = wtmp_pool.tile([P, 4 * P], av_dt, tag="wtmp")
                    nc.vector.tensor_copy(wt[:, 0:gn * P], pt[:, 0:gn * P])
                    for gi in range(gn):
                        nc.tensor.matmul(
                            cps, v_sb[:, g0 + gi, h * D:(h + 1) * D],
                            wt[:, gi * P:(gi + 1) * P],
                            start=(g0 + gi == 0), stop=(g0 + gi == nkc - 1))
                dst = ctx01[h * D:(h + 1) * D, t * P:(t + 1) * P] if h < 2 \
                    else ctx2[0:D, t * P:(t + 1) * P]
                nc.scalar.copy(dst, cps)

    # ---------------- phase 2.5: AV matmuls (dense, deferred) ----------
    for h in (range(HPG) if defer_av else ()):
        for qg in range(0, LQ * P, 512):          # <=512-wide q column groups
            gw = min(512, LQ * P - qg)
            tg0 = qg // P                          # first local tile in group
            ntg = gw // P                          # tiles in this group
            cps = psctx.tile([D, 512], f32, tag="psctx")
            kmax = 2 * (tg0 + ntg - 1) + 1         # last chunk any tile needs
            for kc in range(kmax + 1):
                # columns of this q-group present in the kc segment
                lo_t = max(tg0, wt_tmin[kc])
                hi_t = tg0 + ntg                    # exclusive
                if lo_t >= hi_t:
                    continue
                col = wt_off[kc] + (lo_t - wt_tmin[kc]) * P
                ncols = (hi_t - lo_t) * P
                dst0 = (lo_t - tg0) * P
                nc.tensor.matmul(cps[:, dst0:dst0 + ncols],
                                 v_sb[:, kc, h * D:(h + 1) * D],
                                 wt_sb[h][:, col:col + ncols],
                                 start=(kc == 0), stop=(kc == kmax))
            dst = ctx01[h * D:(h + 1) * D, qg:qg + gw] if h < 2 \
                else ctx2[0:D, qg:qg + gw]
            nc.scalar.copy(dst, cps[:, 0:gw])

    # ---------------- phase 3: output projection ----------------
    for t in range(LQ):
        psy = ps512.tile([P, 1024], f32, tag="ps512", name="psy")
        for od in range(0, HID, 512):
            w = min(512, HID - od)
            nc.tensor.matmul(psy[:, od:od + w],
                             ctx01[:, t * P:(t + 1) * P],
                             wo1_sb[:, od:od + w], start=True, stop=False)
            nc.tensor.matmul(psy[:, od:od + w],
                             ctx2[:, t * P:(t + 1) * P],
                             wo2_sb[:, od:od + w], start=False, stop=True)
        yb = y_pool.tile([P, HID], f32, tag="ybuf")
        nc.vector.tensor_copy(yb, psy[:, 0:HID])
        nc.sync.dma_start(out=y_o[t * P:(t + 1) * P, :], in_=yb)


# ======================= host side =======================

def _sigmoid(x):
    return 1.0 / (1.0 + np.exp(-x))


def _shard_inputs(x, s_prev, Wq, Wk, Wv, Wo, alpha, gamma):
    """Build the 8 per-core input dicts."""
    x2 = np.ascontiguousarray(x.reshape(T, HID))
    xt_full = np.ascontiguousarray(x2.T)                    # [HID, T]
    sqd = np.float32(1.0 / np.sqrt(D))
    in_maps = []
    LQ = T // P // 2
    for c in range(N_CORES):
        g, j = c // 2, c % 2
        hs = slice(g * HPG, (g + 1) * HPG)
        cols = slice(g * HPG * D, (g + 1) * HPG * D)
        # local q rows: global tiles 2t+j
        spg = s_prev[0, hs].reshape(HPG, LQ, 2, P, T)[:, :, j]   # [3, 8, 128, T]
        sp_c = np.ascontiguousarray(spg).reshape(HPG, LQ * P, T)
        xq = x2.reshape(LQ, 2, P, HID)[:, j].reshape(LQ * P, HID)
        xqt = np.ascontiguousarray(xq.T)
        qrel_v = (j * P + np.arange(P, dtype=np.float32)).reshape(P, 1)
        in_maps.append({
            "xt": xt_full,
            "xqt": xqt,
            "wq": np.ascontiguousarray(Wq[:, cols]) * sqd,
            "wk": np.ascontiguousarray(Wk[:, cols]),
            "wv": np.ascontiguousarray(Wv[:, cols]),
            "wo": np.ascontiguousarray(Wo[cols, :]),
            "sp": sp_c,
            "alph": alpha[hs].reshape(1, HPG).astype(np.float32),
            "gamm": gamma[hs].reshape(1, HPG).astype(np.float32),
            "qrel": qrel_v,
            "kio": np.arange(2 * P, dtype=np.float32),
        })
    return in_maps


def _unshard(results):
    LQ = T // P // 2
    raw = np.empty((1, H, T, T), np.float32)
    y = np.zeros((1, T, HID), np.float32)
    for c in range(N_CORES):
        g, j = c // 2, c % 2
        r = results[c]["raw_o"].reshape(HPG, LQ, P, T)
        raw[0, g * HPG:(g + 1) * HPG].reshape(HPG, LQ, 2, P, T)[:, :, j] = r
        yv = y[0].reshape(LQ, 2, P, HID)[:, j]
        yv += results[c]["y_o"].reshape(LQ, P, HID)
    return y, raw


class _Runner:
    """Persistent-jit shard_map executor (axon/PJRT path)."""

    def __init__(self, nc):
        import jax
        from jax.sharding import Mesh, PartitionSpec
        from jax.experimental.shard_map import shard_map
        from concourse import bass2jax
        import concourse.mybir as mybir

        bass2jax.install_neuronx_cc_hook()
        self.jax = jax
        pname = nc.partition_id_tensor.name if nc.partition_id_tensor else None
        in_names, out_names, out_avals = [], [], []
        for alloc in nc.m.functions[0].allocations:
            if not isinstance(alloc, mybir.MemoryLocationSet):
                continue
            name = alloc.memorylocations[0].name
            if alloc.kind == "ExternalInput":
                if name != pname:
                    in_names.append(name)
            elif alloc.kind == "ExternalOutput":
                out_avals.append(jax.core.ShapedArray(
                    tuple(alloc.tensor_shape), mybir.dt.np(alloc.dtype)))
                out_names.append(name)
        self.in_names, self.out_names, self.out_avals = \
            in_names, out_names, out_avals
        n_params, n_outs = len(in_names), len(out_names)
        all_in = list(in_names) + list(out_names)
        if pname is not None:
            all_in.append(pname)

        def _body(*args):
            operands = list(args)
            if pname is not None:
                operands.append(bass2jax.partition_id_tensor())
            return tuple(bass2jax._bass_exec_p.bind(
                *operands, out_avals=tuple(out_avals),
                in_names=tuple(all_in), out_names=tuple(out_names),
                lowering_input_output_aliases=(),
                sim_require_finite=True, sim_require_nnan=True, nc=nc))

        devices = jax.devices()[:N_CORES]
        mesh = Mesh(np.asarray(devices), ("core",))
        specs = (PartitionSpec("core"),)
        self.fn = jax.jit(
            shard_map(_body, mesh=mesh,
                      in_specs=specs * (n_params + n_outs),
                      out_specs=specs * n_outs, check_rep=False),
            keep_unused=True)
        self.n_params = n_params
        self.zero_shapes = [tuple(a.shape) for a in out_avals]
        self.zero_dtypes = [a.dtype for a in out_avals]

    def prepare(self, in_maps):
        n = N_CORES
        concat_in = [
            np.concatenate([np.asarray(m[nm]) for m in in_maps], axis=0)
            for nm in self.in_names]
        concat_zero = [np.zeros((n * s[0], *s[1:]), d)
                       for s, d in zip(self.zero_shapes, self.zero_dtypes)]
        return [self.jax.device_put(a) for a in concat_in + concat_zero]

    def run(self, args):
        outs = self.fn(*args)
        self.jax.block_until_ready(outs)
        return outs

    def __call__(self, in_maps):
        n = N_CORES
        outs = self.run(self.prepare(in_maps))
        return [
            {nm: np.asarray(outs[i]).reshape(n, *self.out_avals[i].shape)[c]
             for i, nm in enumerate(self.out_names)}
            for c in range(n)]


def _get_runner():
    if "runner" not in _cache:
        nc = _build_program()
        _cache["runner"] = _Runner(nc)
    return _cache["runner"]


def measure_hw_ns(np_inputs, n_lo=17, n_hi=257, calls=8):
    """Device-side per-pass time from the slope between two in-NEFF
    For_i loop variants: (wall(n_hi) - wall(n_lo)) / (n_hi - n_lo)."""
    import time
    in_maps = _shard_inputs(
        np.asarray(np_inputs["x"], np.float32),
        np.asarray(np_inputs["s_prev"], np.float32),
        np.asarray(np_inputs["Wq"], np.float32),
        np.asarray(np_inputs["Wk"], np.float32),
        np.asarray(np_inputs["Wv"], np.float32),
        np.asarray(np_inputs["Wo"], np.float32),
        np.asarray(np_inputs["alpha"], np.float32),
        np.asarray(np_inputs["gamma"], np.float32))

    def _timed(runner):
        args = runner.prepare(in_maps)
        runner.run(args)  # warm
        best = float("inf")
        for _ in range(calls):
            t0 = time.perf_counter()
            runner.run(args)
            best = min(best, time.perf_counter() - t0)
        return best

    walls = {}
    for n in (n_lo, n_hi):
        key = f"runner_loop{n}"
        if key not in _cache:
            _cache[key] = _Runner(_build_program(n_loop=n))
        walls[n] = _timed(_cache[key])
        print(f"  wall {n}x: {walls[n]*1e3:.2f} ms")
    return (walls[n_hi] - walls[n_lo]) / (n_hi - n_lo) * 1e9


def kernel(x, s_prev, Wq, Wk, Wv, Wo, alpha, gamma):
    x = np.asarray(x, np.float32)
    s_prev = np.asarray(s_prev, np.float32)
    in_maps = _shard_inputs(x, s_prev,
                            np.asarray(Wq, np.float32),
                            np.asarray(Wk, np.float32),
                            np.asarray(Wv, np.float32),
                            np.asarray(Wo, np.float32),
                            np.asarray(alpha, np.float32),
                            np.asarray(gamma, np.float32))
    results = _get_runner()(in_maps)
    return _unshard(results)


# revision 35
# speedup vs baseline: 2.2745x; 1.5752x over previous
"""Trainium2 Bass kernel for nn_Attention_70514773066385.

Attention with score-history gating bias:
    Q,K,V = x@Wq, x@Wk, x@Wv  (per head)
    raw   = QK^T/sqrt(D) + sigmoid(alpha)*gamma*score_norm(s_prev)
    y     = softmax(raw + causal_mask) @ V @ Wo
    returns (y, raw)

Sharding over 8 NeuronCores: 4 head-groups (3 heads each, Wq/Wk/Wv
column-sharded, Wo row-sharded) x 2 interleaved query-tile sets
(even/odd 128-row tiles, which balances causal-triangle work and makes
the static program identical on every core).  Each core computes raw
and a partial y for its q rows; the host sums the 4 group partials.

Device layouts (T=2048, HID=768, D=64, 3 heads/core, 8 local q-tiles):
  - x arrives pre-transposed (xt [HID,T]) so projections produce
    Q^T/K^T directly ([d, tokens], contraction dim on partitions).
  - scores S = Q^T.T @ K^T accumulate in PSUM [q=128, k=512] tiles;
    evac fused with the gated-norm add (raw = S + A*s_prev + B, A/B
    per-row affine from bn_stats mean/var).
  - softmax without max-subtraction (raw is bounded): exp on ACT with
    accum_out giving the row-sum; causal mask is data-driven
    (k_rel > q_rel -> -1e4) so one program serves both q-tile sets.
  - w rows are normalized then PE-transposed per 128-chunk for the
    AV matmul (ctx^T accumulates over k chunks); y = ctx @ Wo.
"""

import numpy as np

H, D, HID, T = 12, 64, 768, 2048
EPS = 1e-6
N_CORES = 8
NG = 4          # head groups (cores 2g, 2g+1)
HPG = H // NG   # heads per group = 3
P = 128         # partitions / q-tile rows
NEG = -1.0e4    # causal-mask additive (exp underflows to exactly 0)

_cache: dict = {}


def _build_program(av_bf16=True, n_loop=1, use_f32r=True, pool_off=True, defer_av=False):
    import concourse.bass as bass
    import concourse.mybir as mybir
    import concourse.tile as tile
    from concourse import bacc

    f32 = mybir.dt.float32
    av_dt = mybir.dt.bfloat16 if av_bf16 else f32

    CK = HID // P            # 6 contraction chunks for projections
    LQ = T // P // 2         # 8 local q tiles per core
    QW = LQ * P              # 1024 q rows per core
    WC = HPG * D             # 192 weight cols per group
    M0 = P                   # first M split (heads 0,1)
    M1 = WC - P              # 64 (head 2)

    nc = bacc.Bacc("TRN2", target_bir_lowering=False, debug=False,
                   num_devices=N_CORES)

    dram = {}
    def din(name, shape):
        dram[name] = nc.dram_tensor(name, shape, f32, kind="ExternalInput").ap()
        return dram[name]
    def dout(name, shape):
        dram[name] = nc.dram_tensor(name, shape, f32, kind="ExternalOutput").ap()
        return dram[name]

    xt = din("xt", [HID, T])
    xqt = din("xqt", [HID, QW])
    wq = din("wq", [HID, WC])      # pre-scaled by 1/sqrt(D) on host
    wk = din("wk", [HID, WC])
    wv = din("wv", [HID, WC])
    wo = din("wo", [WC, HID])
    sp = din("sp", [HPG, QW, T])
    alph = din("alph", [1, HPG])
    gamm = din("gamm", [1, HPG])
    qrel = din("qrel", [P, 1])     # j*128 + row   (fp32)
    kio = din("kio", [2 * P])      # arange(256)   (fp32)
    raw_o = dout("raw_o", [HPG, QW, T])
    y_o = dout("y_o", [QW, HID])

    import contextlib
    with tile.TileContext(nc) as tc:
        with contextlib.ExitStack() as pctx:
            pools = {
                name: pctx.enter_context(
                    tc.tile_pool(name=name, bufs=bufs, space=space))
                for name, bufs, space in [
                    ("singles", 1, "SBUF"), ("share", 3, "SBUF"),
                    ("ps512", 2, "PSUM"),
                    ("psT", 2, "PSUM"), ("psctx", 2, "PSUM"),
                    ("sp", 5, "SBUF"), ("raw", 2, "SBUF"),
                    ("xq", 2, "SBUF"),
                    ("norm", 2, "SBUF"), ("wrow", 2, "SBUF"),
                    ("wtmp", 3, "SBUF"), ("stat", 4, "SBUF"),
                    ("ybuf", 2, "SBUF"),
                ]}
            if n_loop > 1:
                ET = mybir.EngineType
                with tc.For_i(0, n_loop, 1,
                              hint_engines=(ET.PE, ET.DVE, ET.Activation,
                                            ET.Pool, ET.SP)):
                    _emit_body(tc, nc, bass, mybir, dram, av_dt, CK, LQ, QW,
                               WC, M0, M1, pools, use_f32r, pool_off,
                               defer_av)
            else:
                _emit_body(tc, nc, bass, mybir, dram, av_dt, CK, LQ, QW, WC,
                           M0, M1, pools, use_f32r, pool_off, defer_av)

    nc.compile()
    return nc


def _emit_body(tc, nc, bass, mybir, dram, av_dt, CK, LQ, QW, WC, M0, M1,
               pools, use_f32r=False, pool_off=False, defer_av=False):
    f32 = mybir.dt.float32
    mm_dt = mybir.dt.float32r if use_f32r else f32
    mmdma = nc.gpsimd if use_f32r else nc.sync
    R = lambda ap: ap
    aff_eng = nc.gpsimd if pool_off else nc.vector
    Alu = mybir.AluOpType
    Act = mybir.ActivationFunctionType
    X = mybir.AxisListType.X

    xt, xqt, wq, wk, wv = (dram["xt"], dram["xqt"], dram["wq"],
                           dram["wk"], dram["wv"])
    wo, sp, alph, gamm = dram["wo"], dram["sp"], dram["alph"], dram["gamm"]
    qrel, kio, raw_o, y_o = (dram["qrel"], dram["kio"], dram["raw_o"],
                             dram["y_o"])

    singles = pools["singles"]
    share_pool = pools["share"]
    ps512 = pools["ps512"]
    psT = pools["psT"]
    psctx = pools["psctx"]
    sp_pool = pools["sp"]
    xq_pool = pools["xq"]
    raw_pool = pools["raw"]
    norm_pool = pools["norm"]
    wrow_pool = pools["wrow"]
    wtmp_pool = pools["wtmp"]
    stat_pool = pools["stat"]
    y_pool = pools["ybuf"]

    # ---------------- phase 0: constants ----------------
    xt_sb = share_pool.tile([P, CK, T], mm_dt, tag="share") if defer_av \
        else singles.tile([P, CK, T], mm_dt, tag="xt")
    for c in range(CK):
        mmdma.dma_start(out=xt_sb[:, c, :], in_=xt[c * P:(c + 1) * P, :])
    wqkv_sb = share_pool.tile([P, CK, 3 * WC], mm_dt, tag="share") if defer_av \
        else singles.tile([P, CK, 3 * WC], mm_dt, tag="wqkv")
    wq_sb = wqkv_sb[:, :, 0:WC]
    wk_sb = wqkv_sb[:, :, WC:2 * WC]
    wv_sb = wqkv_sb[:, :, 2 * WC:3 * WC]
    for c in range(CK):
        mmdma.dma_start(out=wq_sb[:, c, :], in_=wq[c * P:(c + 1) * P, :])
        mmdma.dma_start(out=wk_sb[:, c, :], in_=wk[c * P:(c + 1) * P, :])
        mmdma.dma_start(out=wv_sb[:, c, :], in_=wv[c * P:(c + 1) * P, :])
    wo1_sb = singles.tile([P, HID], av_dt, tag="wo1")
    wo2_sb = singles.tile([M1, HID], av_dt, tag="wo2")
    wo_dma = nc.gpsimd if av_dt != f32 else nc.sync
    wo_dma.dma_start(out=wo1_sb, in_=wo[0:P, :])
    wo_dma.dma_start(out=wo2_sb, in_=wo[P:WC, :])

    # causal helpers
    qrel_sb = singles.tile([P, 1], f32, tag="qrel")
    nc.sync.dma_start(out=qrel_sb, in_=qrel)
    kio_sb = norm_pool.tile([P, 2 * P], f32, tag="norm")
    kio_b = bass.AP(tensor=kio.tensor, offset=kio.offset,
                    ap=[[0, P], [1, 2 * P]])
    nc.sync.dma_start(out=kio_sb, in_=kio_b)
    mneg_sb = singles.tile([P, 2 * P], f32, tag="mneg")
    # (kio > qrel) * NEG  -> additive causal mask for the diagonal pair
    nc.vector.tensor_scalar(out=mneg_sb, in0=kio_sb, scalar1=qrel_sb,
                            scalar2=NEG, op0=Alu.is_gt, op1=Alu.mult)

    # gate g_h = sigmoid(alpha_h) * gamma_h, broadcast to [P,1] per head
    ga_sb = singles.tile([1, HPG], f32, tag="ga")
    gm_sb = singles.tile([1, HPG], f32, tag="gm")
    nc.sync.dma_start(out=ga_sb, in_=alph)
    nc.sync.dma_start(out=gm_sb, in_=gamm)
    gv_sb = singles.tile([1, HPG], f32, tag="gv")
    nc.scalar.activation(out=gv_sb, in_=ga_sb, func=Act.Sigmoid)
    nc.vector.tensor_mul(gv_sb, gv_sb, gm_sb)
    gb_sb = singles.tile([P, HPG], f32, tag="gb")
    for h in range(HPG):
        nc.gpsimd.partition_broadcast(gb_sb[:, h:h + 1], gv_sb[0:1, h:h + 1])

    eps_sb = singles.tile([P, 1], f32, tag="eps")
    nc.vector.memset(eps_sb, EPS)
    ident = singles.tile([P, P], av_dt, tag="ident")
    from concourse.masks import make_identity
    make_identity(nc, ident)

    # ---------------- phase 1: projections ----------------
    # Q^T/K^T: out[d_cols, tokens] = W.T @ x^T ; M splits [0:128],[128:192]
    qt01 = singles.tile([P, QW], mm_dt, tag="qt01")
    qt2 = singles.tile([M1, QW], mm_dt, tag="qt2")
    kt01 = singles.tile([P, T], mm_dt, tag="kt01")
    kt2 = singles.tile([M1, T], mm_dt, tag="kt2")

    def project_T(w_sb, x_sb, width, out01, out2):
        for n0 in range(0, width, 512):
            nw = min(512, width - n0)
            for mi, (mof, msz, outt) in enumerate(
                    [(0, M0, out01), (M0, M1, out2)]):
                ps = ps512.tile([P, 512], f32, tag="ps512")
                for c in range(CK):
                    nc.tensor.matmul(
                        ps[0:msz, 0:nw], R(w_sb[:, c, mof:mof + msz]),
                        R(x_sb[:, c, n0:n0 + nw]),
                        start=(c == 0), stop=(c == CK - 1))
                nc.scalar.copy(outt[:, n0:n0 + nw], ps[0:msz, 0:nw])

    # Q^T projection streams xqt by hid-chunk (keeps SBUF small);
    # both M splits accumulate in persistent PSUM tiles across chunks.
    psq0 = ps512.tile([P, 1024], f32, tag="ps512", name="psq0")
    psq1 = ps512.tile([P, 1024], f32, tag="ps512", name="psq1")
    for c in range(CK):
        xq_c = xq_pool.tile([P, QW], mm_dt, tag="xq")
        mmdma.dma_start(out=xq_c, in_=xqt[c * P:(c + 1) * P, :])
        for n0 in range(0, QW, 512):
            nw = min(512, QW - n0)
            nc.tensor.matmul(psq0[:, n0:n0 + nw], R(wq_sb[:, c, 0:M0]),
                             R(xq_c[:, n0:n0 + nw]),
                             start=(c == 0), stop=(c == CK - 1))
            nc.tensor.matmul(psq1[0:M1, n0:n0 + nw],
                             R(wq_sb[:, c, M0:M0 + M1]),
                             R(xq_c[:, n0:n0 + nw]),
                             start=(c == 0), stop=(c == CK - 1))
    for n0 in range(0, QW, 512):
        nw = min(512, QW - n0)
        nc.scalar.copy(qt01[:, n0:n0 + nw], psq0[:, n0:n0 + nw])
        nc.scalar.copy(qt2[:, n0:n0 + nw], psq1[0:M1, n0:n0 + nw])
    project_T(wk_sb, xt_sb, T, kt01, kt2)

    # V: natural layout [tokens, d] ; lhsT = xt chunk (stationary).
    # Emission is spread into the t loop (two token tiles per iteration)
    # so phase-1 PE work does not cork the s_prev/score pipeline.
    v_sb = singles.tile([P, T // P, WC], av_dt, tag="v")

    def emit_v(tk):
        ps = ps512.tile([P, 1024], f32, tag="ps512", name="psv")
        for c in range(CK):
            nc.tensor.matmul(ps[:, 0:WC],
                             R(xt_sb[:, c, tk * P:(tk + 1) * P]),
                             R(wv_sb[:, c, :]),
                             start=(c == 0), stop=(c == CK - 1))
        nc.scalar.copy(v_sb[:, tk, :], ps[:, 0:WC])

    # ---------------- phase 2: attention ----------------
    ctx01 = singles.tile([P, QW], av_dt, tag="ctx01")
    ctx2 = singles.tile([M1, QW], av_dt, tag="ctx2")

    # ragged store for transposed-normalized w chunks: for k-chunk kc the
    # q tiles t >= tmin(kc) participate (tile t covers chunks <= 2t+1)
    NKC = T // P
    wt_tmin = [kc // 2 for kc in range(NKC)]
    wt_off = []
    _o = 0
    for kc in range(NKC):
        wt_off.append(_o)
        _o += (LQ - wt_tmin[kc]) * P
    wt_sb = [share_pool.tile([P, _o], av_dt, tag="share", name=f"wt{h}")
             for h in range(HPG)] if defer_av else None

    for t in range(LQ):
        nkc = 2 * t + 2            # k chunks up to & incl. diagonal pair
        dg0 = 2 * t * P            # diagonal pair column offset
        emit_v(2 * t)
        emit_v(2 * t + 1)

        # stats for all heads of this tile first (one Sqrt -> one LUT swap)
        nsg = max(1, T // 512)
        sp_ts = []
        mv3 = stat_pool.tile([P, HPG, 2], f32, tag="mv3")
        for h in range(HPG):
            sp_t = sp_pool.tile([P, T], f32, tag="sp")
            nc.sync.dma_start(out=sp_t, in_=sp[h, t * P:(t + 1) * P, :])
            sp_ts.append(sp_t)
            stats = stat_pool.tile([P, nsg, 6], f32, tag="stats")
            for s4 in range(nsg):
                nc.vector.bn_stats(out=stats[:, s4, :],
                                   in_=sp_t[:, s4 * 512:(s4 + 1) * 512])
            nc.vector.bn_aggr(out=mv3[:, h, :], in_=stats)
        sd3 = stat_pool.tile([P, HPG], f32, tag="sd3")
        nc.scalar.activation(out=sd3, in_=mv3[:, :, 1], func=Act.Sqrt,
                             bias=eps_sb, scale=1.0)
        A3 = stat_pool.tile([P, HPG], f32, tag="A3")
        nc.vector.reciprocal(A3, sd3)
        nc.vector.tensor_mul(A3, A3, gb_sb)
        B3 = stat_pool.tile([P, HPG], f32, tag="B3")
        nc.vector.tensor_mul(B3, mv3[:, :, 0], A3)
        nc.vector.tensor_scalar_mul(B3, B3, -1.0)

        for h in range(HPG):
            sp_t = sp_ts[h]
            A = A3[:, h:h + 1]
            B = B3[:, h:h + 1]
            # scores + gated norm -> raw
            if h < 2:
                lhsT = qt01[h * D:(h + 1) * D, t * P:(t + 1) * P]
                kth = kt01
                kof = h * D
            else:
                lhsT = qt2[0:D, t * P:(t + 1) * P]
                kth = kt2
                kof = 0
            raw_t = raw_pool.tile([P, T], f32, tag="raw")
            for kb in range(0, T, 1024):
                kw = min(1024, T - kb)
                pss = ps512.tile([P, 1024], f32, tag="ps512")
                for ks in range(kb, kb + kw, 512):
                    nc.tensor.matmul(pss[:, ks - kb:ks - kb + 512], R(lhsT),
                                     R(kth[kof:kof + D, ks:ks + 512]),
                                     start=True, stop=True)
                nt = norm_pool.tile([P, 1024], f32, tag="norm")
                aff_eng.tensor_scalar(out=nt[:, 0:kw],
                                      in0=sp_t[:, kb:kb + kw],
                                      scalar1=A, scalar2=B,
                                      op0=Alu.mult, op1=Alu.add)
                nc.vector.tensor_add(raw_t[:, kb:kb + kw], pss[:, 0:kw],
                                     nt[:, 0:kw])
            nc.sync.dma_start(out=raw_o[h, t * P:(t + 1) * P, :], in_=raw_t)

            # exp (+ causal mask on the diagonal pair), denominator
            wrow = wrow_pool.tile([P, T], av_dt, tag="wrow")
            den_p = stat_pool.tile([P, 6], f32, tag="denp")
            nsp = 0
            for c0 in range(0, dg0, 512):
                w = min(512, dg0 - c0)
                nc.scalar.activation(out=wrow[:, c0:c0 + w],
                                     in_=raw_t[:, c0:c0 + w], func=Act.Exp,
                                     accum_out=den_p[:, nsp:nsp + 1])
                nsp += 1
            dgm = raw_pool.tile([P, 2 * P], f32, tag="dgm")
            nc.vector.tensor_add(dgm, raw_t[:, dg0:dg0 + 2 * P], mneg_sb)
            nc.scalar.activation(out=wrow[:, dg0:dg0 + 2 * P], in_=dgm,
                                 func=Act.Exp, accum_out=den_p[:, nsp:nsp + 1])
            nsp += 1
            den = stat_pool.tile([P, 1], f32, tag="den")
            nc.vector.tensor_reduce(out=den, in_=den_p[:, 0:nsp], axis=X,
                                    op=Alu.add)
            rec = stat_pool.tile([P, 1], f32, tag="rec")
            nc.vector.reciprocal(rec, den)
            nc.vector.tensor_scalar_mul(wrow[:, 0:nkc * P],
                                        wrow[:, 0:nkc * P], rec)

            if defer_av:
                # transpose w chunks into the per-head ragged store
                for kc in range(nkc):
                    pt = psT.tile([P, P], av_dt, tag="psT")
                    nc.tensor.transpose(pt, wrow[:, kc * P:(kc + 1) * P],
                                        ident)
                    col = wt_off[kc] + (t - wt_tmin[kc]) * P
                    nc.vector.tensor_copy(wt_sb[h][:, col:col + P], pt)
            else:
                # transpose in groups of <=4 chunks into one PSUM tile,
                # evacuate with a single wide DVE copy, then AV
                cps = psctx.tile([D, P], f32, tag="psctx")
                for g0 in range(0, nkc, 4):
                    gn = min(4, nkc - g0)
                    pt = psT.tile([P, 4 * P], av_dt, tag="psT")
                    for gi in range(gn):
                        nc.tensor.transpose(
                            pt[:, gi * P:(gi + 1) * P],
                            wrow[:, (g0 + gi) * P:(g0 + gi + 1) * P], ident)
                    wt = wtmp_pool.tile([P, 4 * P], av_dt, tag="wtmp")
                    if (g0 // 4) % 2 == 0:
                        nc.vector.tensor_copy(wt[:, 0:gn * P], pt[:, 0:gn * P])
                    else:
                        nc.scalar.copy(wt[:, 0:gn * P], pt[:, 0:gn * P])
                    for gi in range(gn):
                        nc.tensor.matmul(
                            cps, v_sb[:, g0 + gi, h * D:(h + 1) * D],
                            wt[:, gi * P:(gi + 1) * P],
                            start=(g0 + gi == 0), stop=(g0 + gi == nkc - 1))
                dst = ctx01[h * D:(h + 1) * D, t * P:(t + 1) * P] if h < 2 \
                    else ctx2[0:D, t * P:(t + 1) * P]
                nc.scalar.copy(dst, cps)

    # ---------------- phase 2.5: AV matmuls (dense, deferred) ----------
    for h in (range(HPG) if defer_av else ()):
        for qg in range(0, LQ * P, 512):          # <=512-wide q column groups
            gw = min(512, LQ * P - qg)
            tg0 = qg // P                          # first local tile in group
            ntg = gw // P                          # tiles in this group
            cps = psctx.tile([D, 512], f32, tag="psctx")
            kmax = 2 * (tg0 + ntg - 1) + 1         # last chunk any tile needs
            for kc in range(kmax + 1):
                # columns of this q-group present in the kc segment
                lo_t = max(tg0, wt_tmin[kc])
                hi_t = tg0 + ntg                    # exclusive
                if lo_t >= hi_t:
                    continue
                col = wt_off[kc] + (lo_t - wt_tmin[kc]) * P
                ncols = (hi_t - lo_t) * P
                dst0 = (lo_t - tg0) * P
                nc.tensor.matmul(cps[:, dst0:dst0 + ncols],
                                 v_sb[:, kc, h * D:(h + 1) * D],
                                 wt_sb[h][:, col:col + ncols],
                                 start=(kc == 0), stop=(kc == kmax))
            dst = ctx01[h * D:(h + 1) * D, qg:qg + gw] if h < 2 \
                else ctx2[0:D, qg:qg + gw]
            nc.scalar.copy(dst, cps[:, 0:gw])

    # ---------------- phase 3: output projection ----------------
    for t in range(LQ):
        psy = ps512.tile([P, 1024], f32, tag="ps512", name="psy")
        for od in range(0, HID, 512):
            w = min(512, HID - od)
            nc.tensor.matmul(psy[:, od:od + w],
                             ctx01[:, t * P:(t + 1) * P],
                             wo1_sb[:, od:od + w], start=True, stop=False)
            nc.tensor.matmul(psy[:, od:od + w],
                             ctx2[:, t * P:(t + 1) * P],
                             wo2_sb[:, od:od + w], start=False, stop=True)
        yb = y_pool.tile([P, HID], f32, tag="ybuf")
        nc.scalar.copy(yb, psy[:, 0:HID])
        nc.sync.dma_start(out=y_o[t * P:(t + 1) * P, :], in_=yb)


# ======================= host side =======================

def _sigmoid(x):
    return 1.0 / (1.0 + np.exp(-x))


def _shard_inputs(x, s_prev, Wq, Wk, Wv, Wo, alpha, gamma):
    """Build the 8 per-core input dicts."""
    x2 = np.ascontiguousarray(x.reshape(T, HID))
    xt_full = np.ascontiguousarray(x2.T)                    # [HID, T]
    sqd = np.float32(1.0 / np.sqrt(D))
    in_maps = []
    LQ = T // P // 2
    for c in range(N_CORES):
        g, j = c // 2, c % 2
        hs = slice(g * HPG, (g + 1) * HPG)
        cols = slice(g * HPG * D, (g + 1) * HPG * D)
        # local q rows: global tiles 2t+j
        spg = s_prev[0, hs].reshape(HPG, LQ, 2, P, T)[:, :, j]   # [3, 8, 128, T]
        sp_c = np.ascontiguousarray(spg).reshape(HPG, LQ * P, T)
        xq = x2.reshape(LQ, 2, P, HID)[:, j].reshape(LQ * P, HID)
        xqt = np.ascontiguousarray(xq.T)
        qrel_v = (j * P + np.arange(P, dtype=np.float32)).reshape(P, 1)
        in_maps.append({
            "xt": xt_full,
            "xqt": xqt,
            "wq": np.ascontiguousarray(Wq[:, cols]) * sqd,
            "wk": np.ascontiguousarray(Wk[:, cols]),
            "wv": np.ascontiguousarray(Wv[:, cols]),
            "wo": np.ascontiguousarray(Wo[cols, :]),
            "sp": sp_c,
            "alph": alpha[hs].reshape(1, HPG).astype(np.float32),
            "gamm": gamma[hs].reshape(1, HPG).astype(np.float32),
            "qrel": qrel_v,
            "kio": np.arange(2 * P, dtype=np.float32),
        })
    return in_maps


def _unshard(results):
    LQ = T // P // 2
    raw = np.empty((1, H, T, T), np.float32)
    y = np.zeros((1, T, HID), np.float32)
    for c in range(N_CORES):
        g, j = c // 2, c % 2
        r = results[c]["raw_o"].reshape(HPG, LQ, P, T)
        raw[0, g * HPG:(g + 1) * HPG].reshape(HPG, LQ, 2, P, T)[:, :, j] = r
        yv = y[0].reshape(LQ, 2, P, HID)[:, j]
        yv += results[c]["y_o"].reshape(LQ, P, HID)
    return y, raw


class _Runner:
    """Persistent-jit shard_map executor (axon/PJRT path)."""

    def __init__(self, nc):
        import jax
        from jax.sharding import Mesh, PartitionSpec
        from jax.experimental.shard_map import shard_map
        from concourse import bass2jax
        import concourse.mybir as mybir

        bass2jax.install_neuronx_cc_hook()
        self.jax = jax
        pname = nc.partition_id_tensor.name if nc.partition_id_tensor else None
        in_names, out_names, out_avals = [], [], []
        for alloc in nc.m.functions[0].allocations:
            if not isinstance(alloc, mybir.MemoryLocationSet):
                continue
            name = alloc.memorylocations[0].name
            if alloc.kind == "ExternalInput":
                if name != pname:
                    in_names.append(name)
            elif alloc.kind == "ExternalOutput":
                out_avals.append(jax.core.ShapedArray(
                    tuple(alloc.tensor_shape), mybir.dt.np(alloc.dtype)))
                out_names.append(name)
        self.in_names, self.out_names, self.out_avals = \
            in_names, out_names, out_avals
        n_params, n_outs = len(in_names), len(out_names)
        all_in = list(in_names) + list(out_names)
        if pname is not None:
            all_in.append(pname)

        def _body(*args):
            operands = list(args)
            if pname is not None:
                operands.append(bass2jax.partition_id_tensor())
            return tuple(bass2jax._bass_exec_p.bind(
                *operands, out_avals=tuple(out_avals),
                in_names=tuple(all_in), out_names=tuple(out_names),
                lowering_input_output_aliases=(),
                sim_require_finite=True, sim_require_nnan=True, nc=nc))

        devices = jax.devices()[:N_CORES]
        mesh = Mesh(np.asarray(devices), ("core",))
        specs = (PartitionSpec("core"),)
        self.fn = jax.jit(
            shard_map(_body, mesh=mesh,
                      in_specs=specs * (n_params + n_outs),
                      out_specs=specs * n_outs, check_rep=False),
            keep_unused=True)
        self.n_params = n_params
        self.zero_shapes = [tuple(a.shape) for a in out_avals]
        self.zero_dtypes = [a.dtype for a in out_avals]

    def prepare(self, in_maps):
        n = N_CORES
        concat_in = [
            np.concatenate([np.asarray(m[nm]) for m in in_maps], axis=0)
            for nm in self.in_names]
        concat_zero = [np.zeros((n * s[0], *s[1:]), d)
                       for s, d in zip(self.zero_shapes, self.zero_dtypes)]
        return [self.jax.device_put(a) for a in concat_in + concat_zero]

    def run(self, args):
        outs = self.fn(*args)
        self.jax.block_until_ready(outs)
        return outs

    def __call__(self, in_maps):
        n = N_CORES
        outs = self.run(self.prepare(in_maps))
        return [
            {nm: np.asarray(outs[i]).reshape(n, *self.out_avals[i].shape)[c]
             for i, nm in enumerate(self.out_names)}
            for c in range(n)]


def _get_runner():
    if "runner" not in _cache:
        nc = _build_program()
        _cache["runner"] = _Runner(nc)
    return _cache["runner"]


def measure_hw_ns(np_inputs, n_lo=17, n_hi=257, calls=8):
    """Device-side per-pass time from the slope between two in-NEFF
    For_i loop variants: (wall(n_hi) - wall(n_lo)) / (n_hi - n_lo)."""
    import time
    in_maps = _shard_inputs(
        np.asarray(np_inputs["x"], np.float32),
        np.asarray(np_inputs["s_prev"], np.float32),
        np.asarray(np_inputs["Wq"], np.float32),
        np.asarray(np_inputs["Wk"], np.float32),
        np.asarray(np_inputs["Wv"], np.float32),
        np.asarray(np_inputs["Wo"], np.float32),
        np.asarray(np_inputs["alpha"], np.float32),
        np.asarray(np_inputs["gamma"], np.float32))

    def _timed(runner):
        args = runner.prepare(in_maps)
        runner.run(args)  # warm
        best = float("inf")
        for _ in range(calls):
            t0 = time.perf_counter()
            runner.run(args)
            best = min(best, time.perf_counter() - t0)
        return best

    walls = {}
    for n in (n_lo, n_hi):
        key = f"runner_loop{n}"
        if key not in _cache:
            _cache[key] = _Runner(_build_program(n_loop=n))
        walls[n] = _timed(_cache[key])
        print(f"  wall {n}x: {walls[n]*1e3:.2f} ms")
    return (walls[n_hi] - walls[n_lo]) / (n_hi - n_lo) * 1e9


def kernel(x, s_prev, Wq, Wk, Wv, Wo, alpha, gamma):
    x = np.asarray(x, np.float32)
    s_prev = np.asarray(s_prev, np.float32)
    in_maps = _shard_inputs(x, s_prev,
                            np.asarray(Wq, np.float32),
                            np.asarray(Wk, np.float32),
                            np.asarray(Wv, np.float32),
                            np.asarray(Wo, np.float32),
                            np.asarray(alpha, np.float32),
                            np.asarray(gamma, np.float32))
    results = _get_runner()(in_maps)
    return _unshard(results)
